# revision 11
# baseline (speedup 1.0000x reference)
"""Trainium2 Bass kernel for nn_Attention_52604759441672.

Dense causal self-attention block (LayerNorm -> QKV -> RoPE -> causal
softmax attention -> output projection) for x of shape (2, 2048, 1024),
16 heads x 64 dim. Sharded over 8 NeuronCores: data parallel over the
2 batches x tensor parallel over 4 head-groups (4 heads each). Each core
computes its batch's LayerNorm, its head-group's QKV projections,
attention, and a partial output projection; the host sums the 4 partial
outputs per batch.

v4 (from the v3 232us baseline; trace showed ACT 71% busy / first exp at
68us / 24% HAM-cold / ~4.5us bubbles at each head-pair boundary):
 - startup: x(0) fetched in 4 quarter DMAs so LN starts ~3us in; wkq on
   the scalar ring concurrently; xn^T transposes moved to the gpsimd
   queue (the sync queue was head-of-line blocked); attention(0) starts
   right after chunk 0's k/q+RoPE; the v-projection, ln(1) and kq(1)
   ride as filler inside attention(0).
 - causal mask folded into the score accumulation on the PE (two
   concurrent row-tiled [64,128] matmuls add -60000*triu via an identity
   moving operand) instead of DVE tensor_mul: the score->exp->ctx chain
   no longer crosses the vector engine.
 - head-pair epilogue (denominator broadcast matmul, reciprocal, ctx
   evacuation, normalize) deferred into the next pair's first slots so
   the next pair's scores+exp issue back-to-back; rbden accumulates in
   work_ps so s_ps double-buffering keeps the score pipeline fed.
 - RoPE rotate-half via 4 partition-offset gpsimd adds (no SBUF-SBUF
   block-swap DMAs) and applied per head-pair (half the latency).
 - PSUM evacuations that sat on the ACT queue (wo ocp halves, cx halves)
   moved to DVE: the ACT does almost nothing but the 80 exps.
"""

import os
import sys

for _p in ("/opt/trn_rl_repo",):
    if _p not in sys.path and os.path.isdir(_p):
        sys.path.insert(0, _p)

import numpy as np
import ml_dtypes

import concourse.bass as bass
import concourse.mybir as mybir
import concourse.tile as tile
from concourse import bacc, bass_utils

F32 = mybir.dt.float32
BF16 = mybir.dt.bfloat16
AF = mybir.ActivationFunctionType
ALU = mybir.AluOpType

N_CORES = 8
N = 2048          # sequence length
DIM = 1024        # model dim
DH = 64           # head dim
HPC = 4           # heads per core
HG = HPC * DH     # head-group width = 256
NT = N // 128     # 16 token tiles
KC = DIM // 128   # 8 contraction chunks
CH = N // 512     # 4 q-chunks of 512
VW = DH + 2       # padded v row: 64 dims + ones col + pad (4B align)
SCALE = DH ** -0.5

_CACHE = {}


def _rope_tables():
    inv_freq = 1.0 / (10000.0 ** (np.arange(0, DH, 2, dtype=np.float64) / DH))
    freqs = np.arange(N, dtype=np.float64)[:, None] * inv_freq[None, :]  # (N, 32)
    cos32 = np.cos(freqs).astype(np.float32).T     # (32, N)
    sin32 = np.sin(freqs).astype(np.float32).T     # (32, N)
    cos64 = np.concatenate([cos32, cos32], axis=0)             # (64, N)
    sin64sh = np.concatenate([sin32, -sin32], axis=0)          # pre-shuffled
    cos128 = np.ascontiguousarray(np.tile(cos64, (2, 1)))      # (128, N)
    sinsh128 = np.ascontiguousarray(np.tile(sin64sh, (2, 1)))
    return cos128, sinsh128


def build_nc():
    nc = bacc.Bacc("TRN2", target_bir_lowering=False, debug=False,
                   enable_asserts=True, num_devices=N_CORES)
    dt = nc.dram_tensor
    d = {
        "x": dt("x", [NT, 128, DIM], BF16, kind="ExternalInput").ap(),
        "wkq": dt("wkq", [DIM, 2 * HG], BF16, kind="ExternalInput").ap(),
        "wv": dt("wv", [DIM, HG], BF16, kind="ExternalInput").ap(),
        "wo": dt("wo", [HG, DIM], BF16, kind="ExternalInput").ap(),
        "tabs": dt("tabs", [128, 2, N], BF16, kind="ExternalInput").ap(),
        "blob": dt("blob", [128, 384], BF16, kind="ExternalInput").ap(),
        "onez": dt("onez", [128, 64], BF16, kind="ExternalInput").ap(),
        "out": dt("out", [N, DIM], BF16, kind="ExternalOutput").ap(),
    }
    with tile.TileContext(nc) as tc:
        _emit(nc, tc, d)
    nc.compile()
    return nc


def _emit(nc, tc, d):
    from contextlib import ExitStack
    ctx = ExitStack()
    with ctx:
        consts = ctx.enter_context(tc.tile_pool(name="consts", bufs=1))
        wpool = ctx.enter_context(tc.tile_pool(name="wpool", bufs=1))
        persist = ctx.enter_context(tc.tile_pool(name="persist", bufs=1))
        xcp = ctx.enter_context(tc.tile_pool(name="xcp", bufs=2))
        xnp = ctx.enter_context(tc.tile_pool(name="xnp", bufs=2))
        lnp = ctx.enter_context(tc.tile_pool(name="lnp", bufs=2))
        rqp = ctx.enter_context(tc.tile_pool(name="rqp", bufs=2))
        cxp = ctx.enter_context(tc.tile_pool(name="cxp", bufs=2))
        kqp = ctx.enter_context(tc.tile_pool(name="kqp", bufs=3))
        tbp = ctx.enter_context(tc.tile_pool(name="tbp", bufs=2))
        ph3 = ctx.enter_context(tc.tile_pool(name="ph3", bufs=3))
        ph3s = ctx.enter_context(tc.tile_pool(name="ph3s", bufs=2))
        ph4 = ctx.enter_context(tc.tile_pool(name="ph4", bufs=2))
        # PSUM: work(2x1) + scores(2x2 banks) + ctx(1x2 banks) = 8 banks
        work_ps = ctx.enter_context(
            tc.tile_pool(name="work_ps", bufs=2, space="PSUM"))
        s_ps = ctx.enter_context(tc.tile_pool(name="s_ps", bufs=2, space="PSUM"))
        ctx_ps = ctx.enter_context(
            tc.tile_pool(name="ctx_ps", bufs=1, space="PSUM"))

        x_chunks = {}

        def _fetch_x(cc, quarters=False):
            x_c = xcp.tile([128, 4, DIM], BF16, name=f"x_c{cc}", tag="x_c")
            xv = d["x"].rearrange("(c t) p f -> p (c t) f", c=NT // 4)
            if quarters:
                for q in range(4):
                    nc.sync.dma_start(out=x_c[:, q:q + 1, :],
                                      in_=xv[:, cc * 4 + q:cc * 4 + q + 1, :])
            else:
                nc.sync.dma_start(out=x_c, in_=xv[:, cc * 4:(cc + 1) * 4, :])
            x_chunks[cc] = x_c

        # startup staging: x(0) quarters on sync so LN(0) starts on the
        # first landed tile; merged weight DMA on the scalar ring in
        # parallel (the ACT queue has nothing else to do this early).
        _fetch_x(0, quarters=True)
        wkq_sb = wpool.tile([128, KC, 2 * HG], BF16)
        nc.scalar.dma_start(out=wkq_sb, in_=d["wkq"].rearrange(
            "(kc p) f -> p kc f", p=128))
        wk_sb = wkq_sb[:, :, 0:HG]
        wq_sb = wkq_sb[:, :, HG:2 * HG]
        tabs_sb = consts.tile([128, 2, N], BF16)
        cos_sb = tabs_sb[:, 0, :]
        sinsh_sb = tabs_sb[:, 1, :]
        blob_sb = consts.tile([128, 384], BF16)
        # blob: [:,0:128] negtriuT (-60000*triu(k=1)); [:,128:256] I128;
        # [0:33,256:384] denominator-select rows.
        sel_sb = blob_sb[0:33, 256:384]
        wv_sb = wpool.tile([128, KC, HG], BF16)
        wo_sb = wpool.tile([128, 2, DIM], BF16)
        ropek = persist.tile([128, 2, N], BF16)
        drowP = persist.tile([33, 512], BF16)
        nc.vector.memset(drowP, 0.0)
        vaug = persist.tile([128, NT, HPC, VW], BF16)

        def _fetch_rest():
            # everything not needed in the first ~15us, ordered by urgency
            nc.sync.dma_start(out=blob_sb, in_=d["blob"])
            nc.scalar.dma_start(out=tabs_sb, in_=d["tabs"])
            nc.scalar.dma_start(out=wv_sb, in_=d["wv"].rearrange(
                "(kc p) f -> p kc f", p=128))
            # ones column of v_aug (softmax denominator via the PE)
            nc.sync.dma_start(
                out=vaug[:, :, :, DH:DH + 1],
                in_=d["onez"].rearrange("p (j h o) -> p j h o", j=NT, h=HPC))
            _fetch_x(1)
            nc.scalar.dma_start(out=wo_sb, in_=d["wo"].rearrange(
                "(c p) f -> p c f", p=128))

        xncs = {}

        def ln_steps(c):
            # LayerNorm for chunk c. rstd via Newton from y0=1 (DVE only, no
            # ACT table set beyond exp's is ever needed). xn^T via DMA xbar
            # on the gpsimd queue (sync is busy; gpsimd has slack).
            x_c = x_chunks.pop(c)
            xnc = xnp.tile([128, KC, 512], BF16, name="xnc", tag="xnc")
            xncs[c] = xnc
            mvc = lnp.tile([128, 4, 2], F32, name="mvc", tag="mvc")
            for b4 in range(4):
                stats = lnp.tile([128, 2, 6], F32, name="stats", tag="stats",
                                 bufs=4)
                nc.vector.bn_stats(out=stats[:, 0, :], in_=x_c[:, b4, 0:512])
                nc.vector.bn_stats(out=stats[:, 1, :], in_=x_c[:, b4, 512:1024])
                nc.vector.bn_aggr(out=mvc[:, b4, :], in_=stats)
                yield
            v = mvc[:, :, 1]
            y = lnp.tile([128, 4], F32, name="y", tag="y")
            nc.vector.tensor_scalar(out=y, in0=v, scalar1=-0.5, scalar2=1.5,
                                    op0=ALU.mult, op1=ALU.add)
            for it in range(2):
                t = lnp.tile([128, 4], F32, name="t", tag="t", bufs=4)
                nc.vector.tensor_mul(t, y, y)
                t2 = lnp.tile([128, 4], F32, name="t2", tag="t2", bufs=4)
                nc.vector.tensor_mul(t2, t, v)
                w = lnp.tile([128, 4], F32, name="w", tag="w", bufs=4)
                nc.vector.tensor_scalar(out=w, in0=t2, scalar1=-0.5,
                                        scalar2=1.5, op0=ALU.mult, op1=ALU.add)
                y2 = lnp.tile([128, 4], F32, name="y2", tag="y2", bufs=4)
                nc.vector.tensor_mul(y2, y, w)
                y = y2
            yield
            for b4 in range(4):
                xn_t = lnp.tile([128, DIM], BF16, name="xn_t", tag="xn_t",
                                bufs=4)
                nc.vector.tensor_scalar(out=xn_t, in0=x_c[:, b4, :],
                                        scalar1=mvc[:, b4, 0:1],
                                        scalar2=y[:, b4:b4 + 1],
                                        op0=ALU.subtract, op1=ALU.mult)
                nc.sync.dma_start(out=xnc[:, :, b4 * 128:(b4 + 1) * 128],
                                  in_=xn_t, transpose=True)
                yield

        rqs = {}

        def kq_steps(c):
            # K and Q projections + RoPE for chunk c.
            cs = slice(c * 512, (c + 1) * 512)
            xnc = xncs[c]
            rq = rqp.tile([128, 2, 512], BF16, name="rq", tag="rq")
            rqs[c] = rq

            def tab2(t_sb):
                # [128, 2, 512] view of a rope table chunk, of-dim stride 0
                return bass.AP(tensor=t_sb.tensor,
                               offset=t_sb.offset + c * 512,
                               ap=[list(t_sb.ap[0]), [0, 2], [1, 512]])
            for kind, w_sb in (("k", wk_sb), ("q", wq_sb)):
                kq2 = kqp.tile([128, 2, 512], BF16, name="kq2", tag="kq")
                for of in range(2):
                    ps = work_ps.tile([128, 512], F32, name=f"ps_{kind}{of}",
                                      tag="work")
                    for kc in range(KC):
                        nc.tensor.matmul(
                            ps, w_sb[:, kc, of * 128:(of + 1) * 128],
                            xnc[:, kc, :], start=(kc == 0),
                            stop=(kc == KC - 1))
                    nc.vector.tensor_copy(kq2[:, of, :], ps)
                    yield
                tb = tbp.tile([128, 2, 512], BF16, name="tb", tag=f"tb{kind}")
                tbs = tbp.tile([128, 2, 512], BF16, name="tbs",
                               tag=f"tbs{kind}")
                dst = rq if kind == "q" else ropek[:, :, cs]
                nc.gpsimd.tensor_mul(dst, kq2, tab2(cos_sb))
                nc.gpsimd.tensor_mul(tb, kq2, tab2(sinsh_sb))
                yield
                # rotate_half: swap 32-row blocks 0<->1, 2<->3 (sign is
                # pre-applied in the sinsh table); DMAs on the gpsimd queue
                # so the mul->swap->add chain stays in one FIFO.
                for g in range(4):
                    nc.gpsimd.dma_start(
                        out=tbs[g * 32:(g + 1) * 32, :, :],
                        in_=tb[(g ^ 1) * 32:((g ^ 1) + 1) * 32, :, :])
                nc.gpsimd.tensor_add(dst, dst, tbs)
                yield

        def v_steps(c):
            # token-major V projection for chunk c
            xnc = xncs.pop(c)
            for b4 in range(4):
                vps = work_ps.tile([128, HG], F32, name=f"vps{b4}", tag="work")
                for kc in range(KC):
                    nc.tensor.matmul(
                        vps, xnc[:, kc, b4 * 128:(b4 + 1) * 128],
                        wv_sb[:, kc, :], start=(kc == 0), stop=(kc == KC - 1))
                nc.vector.tensor_copy(
                    vaug[:, c * 4 + b4, :, 0:DH],
                    vps.rearrange("p (h dd) -> p h dd", h=HPC))
                yield

        def wo_steps(c, cx):
            # output projection for token tiles of chunk c
            for b4 in range(4):
                it = c * 4 + b4
                ocp = ph4.tile([128, 2, 512], BF16, name="ocp", tag="ocp")
                for nh in range(2):
                    op = work_ps.tile([128, 512], F32, name="op", tag="work")
                    for pc in range(2):
                        nc.tensor.matmul(
                            op, cx[:, pc, b4 * 128:(b4 + 1) * 128],
                            wo_sb[:, pc, nh * 512:(nh + 1) * 512],
                            start=(pc == 0), stop=(pc == 1))
                    nc.vector.tensor_copy(ocp[:, nh, :], op)
                    yield
                nc.sync.dma_start(
                    out=d["out"][it * 128:(it + 1) * 128, :],
                    in_=ocp.rearrange("p a f -> p (a f)"))

        def epilogue(c, p, ctx2, pend, cx):
            # final ctx accumulation + softmax denominators + ctx
            # evacuation for head-pair (c, p). Runs interleaved with the
            # NEXT pair's first score slots.
            pj, pats, plo = pend
            for hi in range(2):
                h = 2 * p + hi
                nc.tensor.matmul(
                    ctx2[:, hi, plo:512], vaug[:, pj, h, 0:DH + 1],
                    pats[:, hi, plo:512], start=(pj == 0), stop=True)
            yield
            # denominators: row DH of the ctx accumulator pair -> partitions
            # 0/32 of drowP -> one K=33 select matmul broadcasts them across
            # the two 64-partition halves -> fast reciprocal.
            nc.scalar.copy(drowP[0:1, :], ctx2[DH:DH + 1, 0, :])
            nc.scalar.copy(drowP[32:33, :], ctx2[DH:DH + 1, 1, :])
            yield
            rbden = work_ps.tile([128, 512], F32, name="rbden", tag="work")
            nc.tensor.matmul(rbden, sel_sb, drowP, start=True, stop=True)
            yield
            rbf = ph3s.tile([128, 512], F32, name="rbf", tag="rbf")
            nc.vector.reciprocal_approx_fast(out=rbf, in_=rbden)
            rb = ph3s.tile([128, 512], BF16, name="rb", tag="rb")
            nc.vector.tensor_copy(rb, rbf)
            yield
            # evacuate ctx (unscaled) then normalize in SBUF via gpsimd
            nc.vector.tensor_copy(cx[0:DH, p, :], ctx2[0:DH, 0, :])
            nc.vector.tensor_copy(cx[DH:128, p, :], ctx2[0:DH, 1, :])
            yield
            nc.gpsimd.tensor_mul(cx[:, p, :], cx[:, p, :], rb)

        def attention(c, filler, pulls_per_slot, pre):
            # causal attention for q-chunk c, both head pairs; `filler`
            # yields independent work interleaved between slots; `pre` is
            # the previous pair's deferred epilogue. Returns (cx, epi) with
            # epi = this chunk's last-pair epilogue, still unemitted.
            def pull(gen, k):
                for _ in range(k):
                    if next(gen, "done") == "done":
                        break
            rq = rqs.pop(c)
            cx = cxp.tile([128, 2, 512], BF16, name="cx", tag="cx")
            nj = 4 * (c + 1)
            epi = pre
            for p in range(2):
                ctx2 = None
                pend = None  # (j, a_t, lo) waiting for its ctx matmuls
                for j in range(nj):
                    dj = j - 4 * c
                    lo = max(dj, 0) * 128  # causally-valid q-column start
                    sp = s_ps.tile([128, 2, 512], F32, name="sp", tag="sp")
                    diag = dj >= 0
                    for hi in range(2):
                        off = hi * DH
                        nc.tensor.matmul(
                            sp[:, hi, lo:512],
                            ropek[off:off + DH, p, j * 128:(j + 1) * 128],
                            rq[off:off + DH, p, lo:512],
                            start=True, stop=not diag,
                            tile_position=(off, 0))
                    if diag:
                        # fold the causal mask into the score accumulation:
                        # out[key, qc] += -60000*[key > qc] via an identity
                        # moving operand (full 128-row mode, one matmul per
                        # PSUM bank, serial -- no cross-tile bank hazards).
                        ntT = blob_sb[:, 0:128]
                        id128 = blob_sb[:, 128:256]
                        nc.tensor.matmul(
                            sp[:, 0, lo:lo + 128], ntT, id128,
                            start=False, stop=True)
                        nc.tensor.matmul(
                            sp[:, 1, lo:lo + 128], ntT, id128,
                            start=False, stop=True)
                    a_t = ph3.tile([128, 2, 512], BF16, name="a_t", tag="a_t")
                    nc.scalar.activation(
                        out=a_t[:, :, lo:512], in_=sp[:, :, lo:512],
                        func=AF.Exp, scale=float(SCALE))
                    if epi is not None:
                        pull(epi, 1000)
                        epi = None
                    if pend is not None:
                        if ctx2 is None:
                            ctx2 = ctx_ps.tile([DH + 1, 2, 512], F32,
                                               name="ctx2", tag="ctx2")
                        pj, pats, plo = pend
                        for hi in range(2):
                            h = 2 * p + hi
                            nc.tensor.matmul(
                                ctx2[:, hi, plo:512],
                                vaug[:, pj, h, 0:DH + 1],
                                pats[:, hi, plo:512],
                                start=(pj == 0), stop=False)
                    pend = (j, a_t, lo)
                    pull(filler, pulls_per_slot)
                epi = epilogue(c, p, ctx2, pend, cx)
            # drain whatever filler remains before leaving the chunk
            pull(filler, 1000)
            return cx, epi

        # ---------------- main schedule ----------------
        import itertools
        from collections import deque

        def roundrobin(*gens):
            q = deque(g for g in gens if g is not None)
            while q:
                g = q.popleft()
                if next(g, "done") != "done":
                    q.append(g)
                    yield

        _fetch_rest()
        for _ in ln_steps(0):
            pass
        for _ in kq_steps(0):
            pass
        # ln(1) overlaps attention(0): its DVE chain runs while the PE does
        # chunk 0's scores, so kq(1) (filler in attention(0)) finds xnc(1)
        # ready. From then on ln runs two chunks ahead.
        for _ in ln_steps(1):
            pass

        cxs = {}
        epi = None
        for c in range(CH):
            if c + 2 < CH:
                _fetch_x(c + 2)
            rr = roundrobin(
                v_steps(c),
                kq_steps(c + 1) if c + 1 < CH else None,
                ln_steps(c + 2) if c + 2 < CH else None)
            parts = [rr]
            if c - 1 >= 0:
                parts.append(wo_steps(c - 1, cxs.pop(c - 1)))
            filler = itertools.chain(*parts)
            pulls = {0: 3, 1: 2, 2: 1, 3: 1}[c]
            cxs[c], epi = attention(c, filler, pulls, epi)
        for _ in epi:
            pass
        for _ in wo_steps(CH - 1, cxs.pop(CH - 1)):
            pass


def make_in_maps(x, gamma, beta, Wq, Wkv, Wo):
    x = np.asarray(x, dtype=np.float32)
    gamma = np.asarray(gamma, dtype=np.float32)
    beta = np.asarray(beta, dtype=np.float32)
    Wq = np.asarray(Wq, dtype=np.float32)
    Wkv = np.asarray(Wkv, dtype=np.float32)
    Wo = np.asarray(Wo, dtype=np.float32)
    if np.any(beta != 0.0):
        raise NotImplementedError("nonzero beta not supported by this kernel")
    bf = ml_dtypes.bfloat16
    wq_f = (gamma[:, None] * Wq).astype(bf)       # fold gamma into weights
    wk_f = (gamma[:, None] * Wkv[:, :DIM]).astype(bf)
    wv_f = (gamma[:, None] * Wkv[:, DIM:]).astype(bf)
    cos128, sinsh128 = _rope_tables()
    tabs = np.stack([cos128, sinsh128], axis=1).astype(bf)  # [128, 2, N]
    blob = np.zeros((128, 384), dtype=np.float32)
    # negtriuT: additive causal mask, applied on the PE via identity matmul
    blob[:, 0:128] = -60000.0 * np.triu(np.ones((128, 128), dtype=np.float32),
                                        k=1)
    blob[:, 128:256] = np.eye(128, dtype=np.float32)
    blob[0, 256 + 0:256 + DH] = 1.0        # sel row 0
    blob[32, 256 + DH:256 + 128] = 1.0     # sel row 32
    xb = x.astype(bf).reshape(2, NT, 128, DIM)
    in_maps = []
    for core in range(N_CORES):
        b, hg = divmod(core, 4)
        sl = slice(hg * HG, (hg + 1) * HG)
        wkq = np.concatenate([wk_f[:, sl], wq_f[:, sl]], axis=1)
        in_maps.append({
            "x": np.ascontiguousarray(xb[b]),
            "wkq": np.ascontiguousarray(wkq),
            "wv": np.ascontiguousarray(wv_f[:, sl]),
            "wo": np.ascontiguousarray(Wo[sl, :].astype(bf)),
            "tabs": np.ascontiguousarray(tabs),
            "blob": blob.astype(bf),
            "onez": np.ones((128, 64), dtype=bf),
        })
    return in_maps


def kernel(x, gamma, beta, Wq, Wkv, Wo, _trace=False):
    in_maps = make_in_maps(x, gamma, beta, Wq, Wkv, Wo)
    if "nc" not in _CACHE:
        _CACHE["nc"] = build_nc()
    nc = _CACHE["nc"]
    res = bass_utils.run_bass_kernel_spmd(
        nc, in_maps, core_ids=list(range(N_CORES)), trace=_trace)
    out = np.zeros((2, N, DIM), dtype=np.float64)
    for core in range(N_CORES):
        b = core // 4
        out[b] += res.results[core]["out"].astype(np.float64)
    _CACHE["last_results"] = res
    return out.astype(np.float32)
